# revision 2
# baseline (speedup 1.0000x reference)
"""Trainium2 Bass kernel for nn_Attention_layer_1580547966657.

Reference computation (B=8, S=2048, E=H=1024):
    q,k,v = x@W{q,k,v}.T + b;  scores = q@k.T/sqrt(H), query-row masked;
    att = softmax(scores) @ v;  out = att @ Wn.T  -> (B, S, 1)
    out = softmax(out, axis=-1)                   -> (B, S, 1)

The final softmax is over an axis of size 1, so the output is exactly
1.0 everywhere for any finite inputs: softmax of a single element is
exp(x-x)/exp(x-x) = 1. All upstream values are finite (finite inputs,
masking uses a large-negative constant, softmax over S is well-defined),
so the whole attention pipeline algebraically cancels out of the output.

The kernel therefore only has to materialize ones((B,S,1), f32) on the
devices: each of the 8 cores handles one batch row (data-parallel over
B), memsets a [128,16] SBUF tile to 1.0 and DMAs its 2048-element output
shard to DRAM.
"""

import numpy as np

import concourse.bass as bass
import concourse.mybir as mybir
from concourse.bass_utils import run_bass_kernel_spmd

B, S = 8, 2048
P, N = 128, S // 128  # per-core output viewed as [128 partitions, 16 elems]

_cache = {}


def _build():
    nc = bass.Bass()
    out = nc.dram_tensor("out", (P, N), mybir.dt.float32, kind="ExternalOutput")
    with (
        nc.sbuf_tensor([P, N], mybir.dt.float32) as tile,
        nc.semaphore() as s_v,
        nc.semaphore() as s_d,
        nc.Block() as block,
    ):

        @block.vector
        def _(vector):
            vector.memset(tile[:], 1.0).then_inc(s_v, 1)

        @block.sync
        def _(sync):
            sync.wait_ge(s_v, 1)
            sync.dma_start(out[:], tile[:]).then_inc(s_d, 16)
            sync.wait_ge(s_d, 16)

    return nc


def kernel(x, mask, Wq, bq, Wk, bk, Wv, bv, Wn):
    if "nc" not in _cache:
        _cache["nc"] = _build()
    nc = _cache["nc"]
    res = run_bass_kernel_spmd(nc, [{} for _ in range(B)], core_ids=list(range(B)))
    return np.stack([r["out"].reshape(S, 1) for r in res.results])


# revision 3
# speedup vs baseline: 51917.8003x; 51917.8003x over previous
"""Trainium2 Bass kernel for nn_Attention_layer_1580547966657.

Reference computation (B=8, S=2048, E=H=1024):
    q,k,v = x@W{q,k,v}.T + b;  scores = q@k.T/sqrt(H), query-row masked;
    att = softmax(scores) @ v;  out = att @ Wn.T  -> (B, S, 1)
    out = softmax(out, axis=-1)                   -> (B, S, 1)

The final softmax is over an axis of size 1, so the output is exactly
1.0 everywhere for any finite inputs: softmax of a single element is
exp(x-x)/exp(x-x) = 1. All upstream values stay finite for any
realistically-scaled finite inputs (masking uses a large-negative
constant, not -inf, and the row softmax over S is max-shifted), so the
whole attention pipeline algebraically cancels out of the output.

The kernel therefore only has to materialize ones((B,S,1), f32):
B=8 is sharded one batch row per core (data-parallel over batch, per the
sharding hint). Each core's program is a single DMA of an embedded
8 KiB constant (ones, loaded to HBM at NEFF load time) to its output
shard, plus the DMA-completion semaphore wait. TimelineSim: ~3.2 us/core
(NRT launch overhead dominates; the payload DMA is <1 us).
"""

import numpy as np

import concourse.bass as bass
import concourse.mybir as mybir
from concourse.bass_utils import run_bass_kernel_spmd

B, S = 8, 2048
P, N = 128, S // 128  # per-core output viewed as [128 partitions, 16 elems]

_cache = {}


def _build():
    nc = bass.Bass()
    out = nc.dram_tensor("out", (P, N), mybir.dt.float32, kind="ExternalOutput")
    ones = nc.inline_tensor(np.ones((P, N), np.float32), name="ones_const")
    s_d = nc.alloc_semaphore("s_d")
    nc.sync.dma_start(out[:], ones[:]).then_inc(s_d, 16)
    nc.sync.wait_ge(s_d, 16)
    return nc


def kernel(x, mask, Wq, bq, Wk, bk, Wv, bv, Wn):
    if "nc" not in _cache:
        _cache["nc"] = _build()
    res = run_bass_kernel_spmd(
        _cache["nc"], [{} for _ in range(B)], core_ids=list(range(B))
    )
    return np.stack([r["out"].reshape(S, 1) for r in res.results])
